# revision 33
# baseline (speedup 1.0000x reference)
"""Trainium2 Bass kernel for nn_MultiHeadAttention (B=2, S=2048, D=1024, H=16,
DK=DV=64, causal mask), sharded over 8 NeuronCores.

Sharding: data-parallel on batch (cores 0-3 -> b=0, cores 4-7 -> b=1) x
tensor-parallel on heads (each core owns 4 heads = 256 cols of Wq/Wk/Wv and
256 rows of Wo). Each core computes a partial output projection; the host sums
the 4 partials per batch, adds bo, and applies q_mask.

All matmuls run in bf16 (1 PE cycle/row vs 4 for fp32; rel-err budget 2e-2
easily covers it), accumulating in fp32 PSUM. Inputs ship as bf16 to halve
DMA. Per-core device pipeline:
  1. QW^T[c,j], KW^T[c,j] via PE (contract D, stationary=weights), bias added
     on the PSUM->SBUF copy (DVE) producing bf16 operand tiles.
  2. VW computed directly in [k, c] layout (stationary = v-chunk k-slice,
     moving = Wv) -- no PE transposes; bias added on the PSUM->SBUF copy into
     vw_aug, which carries a ones column per head (row-sum trick).
  3. Per head: scores A^T[k,j] = KW^T.T @ QW^T on PE; diagonal tiles are
     column-restricted to j >= 128d and masked with one triangular [128,128]
     DVE add; exp via ACT (scale=1/8 folded in, no max-subtraction -- logits
     are O(6)) writing bf16; O^T_aug[65,j] = [VW|1].T @ P^T accumulated over
     k; row 64 = softmax denominators. Upper-triangular tile-blocks are
     skipped entirely. Depth-4 software pipeline on the PE so it never idles
     on ACT's exp latency; the next chunk's projections interleave before the
     output projection so the last head's normalize hides under them.
  4. Normalize: reciprocal (DVE) -> rank-1 PE matmul (ones x recip) to
     broadcast across partitions -> multiply on the PSUM->SBUF copy into
     head-PAIR O^T buffers (128 partitions: head 2g in rows 0-63, head 2g+1
     in rows 64-127).
  5. Output projection with per-PAIR K=128 c-tiles (full PE height) ->
     partial [2048,1024] bf16, written via two-row-group batched DMAs.
"""
import numpy as np
import ml_dtypes

import concourse.bass as bass
import concourse.mybir as mybir
from concourse.tile import TileContext
from concourse import bass2jax

# ---- problem constants (hardcoded per contract) ----
B, S, D = 2, 2048, 1024
H, DK, DV = 16, 64, 64
NCORES = 8
GROUPS = NCORES // B          # cores per batch = 4
HC = H // GROUPS              # heads per core = 4
CW = HC * DK                  # per-core width = 256
P = 128                       # partitions
JC = 512                      # j-chunk (moving free dim)
NJC = S // JC                 # 4
NKT = S // P                  # 16 k-tiles
NDT = D // P                  # 8 D-tiles
MASKVAL = -8.0e4              # pre-scale additive mask (=> logit -1e4)

f32 = mybir.dt.float32
bf16 = mybir.dt.bfloat16
npbf16 = ml_dtypes.bfloat16

_CACHE = {}


def _legalize_waits(nc, max_waits=1):
    """This walrus build accepts at most one on_wait per instruction; hoist
    extras onto same-engine NOPs inserted immediately before."""
    import bass_rust
    n = 0
    for f in nc.m.functions:
        for bb in f.blocks:
            insts = bb.instructions
            if not any(
                (inst.sync_info is not None and len(inst.sync_info.on_wait) > max_waits)
                for inst in insts
            ):
                continue
            out = []
            for inst in insts:
                si = inst.sync_info
                if si is not None and len(si.on_wait) > max_waits:
                    waits = list(si.on_wait)
                    for w in waits[:-max_waits]:
                        nop = mybir.InstNoOp(name=f"lwnop-{n}")
                        n += 1
                        nop.engine = inst.engine
                        nop.sync_info = bass_rust.SyncInfo(on_wait=[w], on_update=[])
                        out.append(nop)
                    inst.sync_info = bass_rust.SyncInfo(
                        on_wait=waits[-max_waits:], on_update=list(si.on_update)
                    )
                out.append(inst)
            bb.instructions = out
    return n


def _build(causal=True, loop_k=None):
    nc = bass.Bass(trn_type="TRN2", target_bir_lowering=False, debug=False)

    qT = nc.dram_tensor("qT", [D, S], bf16, kind="ExternalInput")
    kT = nc.dram_tensor("kT", [D, S], bf16, kind="ExternalInput")
    vT = nc.dram_tensor("vT", [D, S], bf16, kind="ExternalInput")
    wqkv = nc.dram_tensor("wqkv", [3, D, CW], bf16, kind="ExternalInput")
    wo = nc.dram_tensor("wo", [CW, D], bf16, kind="ExternalInput")
    bqk = nc.dram_tensor("bqk", [2, 2, P], f32, kind="ExternalInput")  # [q/k, hp, d]
    bvv = nc.dram_tensor("bv", [CW], f32, kind="ExternalInput")
    masks = nc.dram_tensor("masks", [P, P], f32, kind="ExternalInput")
    amask = None
    if not causal:
        amask = nc.dram_tensor("amask", [S, S], f32, kind="ExternalInput")
    out = nc.dram_tensor("out", [S, D], bf16, kind="ExternalOutput")

    def ktiles_for(jc):
        return NKT if not causal else 4 * jc + 4

    with TileContext(nc) as tc:
        with tc.tile_pool(name="const", bufs=1) as const, \
             tc.tile_pool(name="chunks", bufs=2) as chunks, \
             tc.tile_pool(name="pt", bufs=6) as ptp, \
             tc.tile_pool(name="small", bufs=4) as small, \
             tc.tile_pool(name="opst", bufs=2) as opst, \
             tc.tile_pool(name="amp", bufs=3) as amp, \
             tc.tile_pool(name="big", bufs=5, space="PSUM") as big, \
             tc.tile_pool(name="av", bufs=2, space="PSUM") as av, \
             tc.tile_pool(name="rbp", bufs=1, space="PSUM") as rbp:

            def emit():
                # ---------- resident tensors; ordered so the critical path
                # (q-weights, then the first chunk) lands first ----------
                # v-weights and the v chunk land first: the per-chunk pipeline
                # starts with the V projection
                wqkv_sb = const.tile([P, 3, NDT, CW], bf16, tag="wqkv")
                for w in (2, 0, 1):
                    nc.scalar.dma_start(
                        out=wqkv_sb[:, w],
                        in_=wqkv.ap().rearrange("w (dt p) c -> p w dt c", p=P)[:, w])

                srcs = (qT, kT, vT)
                ch_tiles = {}

                def load_chunks(jc, split_v=False):
                    tiles = {}
                    for w in (2, 0, 1):
                        ch = chunks.tile([P, NDT, JC], bf16, tag=f"chunk{w}",
                                         name=f"ch{w}_{jc}")
                        src = srcs[w].ap().rearrange("(dt p) s -> p dt s", p=P) \
                            [:, :, bass.ts(jc, JC)]
                        if w == 2 and split_v:
                            # split so the first V-projection matmul (dt=0) can
                            # start after a quarter of the transfer
                            nc.sync.dma_start(out=ch[:, 0:2], in_=src[:, 0:2])
                            nc.sync.dma_start(out=ch[:, 2:], in_=src[:, 2:])
                        else:
                            nc.sync.dma_start(out=ch, in_=src)
                        tiles[w] = ch
                    ch_tiles[jc] = [tiles[0], tiles[1], tiles[2]]

                load_chunks(0, split_v=True)

                bqk_sb = const.tile([P, 2, 2], f32, tag="bqk")
                nc.scalar.dma_start(out=bqk_sb, in_=bqk.ap().rearrange("qk hp p -> p qk hp"))
                bv_sb = const.tile([P, CW], f32, tag="bv")
                nc.scalar.dma_start(out=bv_sb,
                                  in_=bass.AP(tensor=bvv, offset=0, ap=[[0, P], [1, CW]]))
                masks_sb = const.tile([P, P], f32, tag="masks")
                nc.scalar.dma_start(out=masks_sb, in_=masks.ap())
                wo2 = const.tile([P, 2, D], bf16, tag="wo2")
                nc.scalar.dma_start(out=wo2, in_=wo.ap().rearrange("(g p) e -> p g e", p=P))

                ones64 = const.tile([1, DV], bf16, tag="ones64")
                nc.vector.memset(ones64, 1.0)

                qwt = [const.tile([P, S], bf16, tag=f"qwt{hp}", name=f"qwt{hp}")
                       for hp in range(2)]
                kwt = [const.tile([P, S], bf16, tag=f"kwt{hp}", name=f"kwt{hp}")
                       for hp in range(2)]
                vw_aug = const.tile([P, NKT, HC, DV + 1], bf16, tag="vw_aug")
                nc.vector.memset(vw_aug[:, :, :, DV:DV + 1], 1.0)
                # head-pair output buffers: pair g holds head 2g (rows 0-63)
                # and head 2g+1 (rows 64-127)
                otp = [const.tile([P, S], bf16, tag=f"otp{g}", name=f"otp{g}")
                       for g in range(2)]

                # ---------- interleaved per j-chunk: proj -> attention -> outproj ----
                # Causality: queries in chunk jc attend only to k-tiles
                # 0..4jc+3, all produced by chunks <= jc, so attention for jc
                # can start right after its projections -- later chunks' DMAs
                # stream underneath.
                def proj(jc, deferred=None):
                    """V projection directly in [k, c] layout (first: its PSUM
                    slots recycle score tiles whose exp readers finished long
                    ago), then Q/K projections to [c, j].  `deferred` is
                    emitted after the second V psum -- used to slot the
                    previous chunk's last normalize under independent PE work."""
                    js = bass.ts(jc, JC)
                    chv = ch_tiles[jc][2]
                    for t in range(JC // P):
                        kt = jc * (JC // P) + t
                        pv = big.tile([P, CW], f32, tag="big", name=f"pv{kt}")
                        for dt in range(NDT):
                            nc.tensor.matmul(pv, chv[:, dt, bass.ts(t, P)],
                                             wqkv_sb[:, 2, dt, :],
                                             start=(dt == 0), stop=(dt == NDT - 1))
                        if t == 1 and deferred is not None:
                            deferred()
                            deferred = None
                        nc.vector.tensor_add(vw_aug[:, kt, :, 0:DV], pv, bv_sb)
                    for w in range(2):
                        ch = ch_tiles[jc][w]
                        dst = qwt if w == 0 else kwt
                        for hp in range(2):
                            ps = big.tile([P, JC], f32, tag="big",
                                          name=f"psp{w}{jc}{hp}")
                            for dt in range(NDT):
                                nc.tensor.matmul(ps, wqkv_sb[:, w, dt, bass.ts(hp, P)],
                                                 ch[:, dt, :],
                                                 start=(dt == 0), stop=(dt == NDT - 1))
                            nc.vector.tensor_scalar_add(dst[hp][:, js], ps,
                                                        bqk_sb[:, w, hp:hp + 1])

                proj(0)
                for jc in range(NJC):
                    js = bass.ts(jc, JC)
                    # prefetch next chunk's inputs ahead of attention/outproj so
                    # they don't queue behind this chunk's output stores
                    del ch_tiles[jc]
                    if jc + 1 < NJC:
                        load_chunks(jc + 1)

                    # --- attention for this query chunk, all heads ---
                    # Per-head normalize is split so `av` only needs 2 PSUM
                    # banks (freeing 2 for deeper score double-buffering):
                    # rcp(h) issues right after head h's last AV (so the DVE
                    # computes it under head h+1's scores), while the PE
                    # rank-1 broadcast + multiply for h are deferred until
                    # after head h+1's first few score matmuls.
                    nkt = ktiles_for(jc)
                    pos = {}
                    rcps = {}

                    def norm_tail(h):
                        po = pos.pop(h)
                        g, hh = divmod(h, 2)
                        rb = rbp.tile([DV, JC], f32, tag="rb", name=f"rb{h}{jc}")
                        nc.tensor.matmul(rb, ones64, rcps.pop(h), start=True,
                                         stop=True, skip_group_check=True)
                        rbs = small.tile([DV, JC], f32, tag="rbs", name=f"rbs{h}{jc}")
                        nc.vector.tensor_copy(out=rbs, in_=rb)
                        nc.vector.tensor_mul(otp[g][hh * DV:(hh + 1) * DV, js],
                                             po[0:DV, :], rbs)

                    for h in range(HC):
                        hp, hh = divmod(h, 2)
                        drow = slice(hh * DV, hh * DV + DV)
                        po = av.tile([DV + 1, JC], f32, tag="av",
                                     name=f"psumo_{h}_{jc}")
                        pos[h] = po
                        # depth-2 software pipeline on the PE: AV(kt) is issued
                        # only after score(kt+2), so the in-order PE never idles
                        # on ACT's exp latency.
                        # Diagonal tiles (kt = 4jc+d) only produce valid scores
                        # for columns j >= 128d; everything (score matmul, mask
                        # add, exp, AV) is restricted to that column range, and
                        # only the first 128 columns of it need the triangular
                        # mask.
                        pending = []   # [(kt, pt, off), ...] awaiting AV matmul
                        for kt in range(nkt):
                            if kt == 3 and h > 0:
                                norm_tail(h - 1)
                            dlt = kt - 4 * jc
                            off = P * dlt if (causal and dlt > 0) else 0
                            ps = big.tile([P, JC], f32, tag="big",
                                          name=f"sc_h{h}_{kt}_{jc}")
                            nc.tensor.matmul(ps[:, off:], kwt[hp][drow, bass.ts(kt, P)],
                                             qwt[hp][drow, jc * JC + off:(jc + 1) * JC],
                                             start=True, stop=True)
                            if causal and 0 <= dlt <= 3:
                                nc.vector.tensor_add(ps[:, off:off + P],
                                                     ps[:, off:off + P], masks_sb)
                            if not causal:
                                am = amp.tile([P, JC], f32, tag="am",
                                              name=f"am_h{h}_{kt}_{jc}")
                                nc.sync.dma_start(
                                    out=am,
                                    in_=amask.ap()[bass.ts(kt, P), bass.ts(jc, JC)])
                                nc.vector.tensor_add(ps, ps, am)
                            pt = ptp.tile([P, JC], bf16, tag="pt",
                                          name=f"pt_h{h}_{kt}_{jc}")
                            nc.scalar.activation(out=pt[:, off:], in_=ps[:, off:],
                                                 func=mybir.ActivationFunctionType.Exp,
                                                 scale=0.125)
                            pending.append((kt, pt, off))
                            if len(pending) > 4:
                                pkt, ppt, poff = pending.pop(0)
                                nc.tensor.matmul(po[:, poff:], vw_aug[:, pkt, h, :],
                                                 ppt[:, poff:],
                                                 start=(pkt == 0), stop=False,
                                                 skip_group_check=True)
                        for pkt, ppt, poff in pending:
                            nc.tensor.matmul(po[:, poff:], vw_aug[:, pkt, h, :],
                                             ppt[:, poff:],
                                             start=(pkt == 0), stop=(pkt == nkt - 1),
                                             skip_group_check=True)
                        rcp = small.tile([1, JC], bf16, tag="rcp", name=f"rcp{h}{jc}")
                        with nc.allow_low_precision(reason="bf16 softmax recip"):
                            nc.vector.reciprocal(rcp, po[DV:DV + 1, :])
                        rcps[h] = rcp

                    # last head's normalize hides under the next chunk's
                    # projection matmuls (independent PE work) so the PE
                    # doesn't wait on the DVE reciprocal/multiply chain
                    if jc + 1 < NJC:
                        proj(jc + 1, deferred=lambda: norm_tail(HC - 1))
                    else:
                        norm_tail(HC - 1)

                    # --- output projection for this chunk's 4 j-tiles ---
                    for t2 in range(2):
                        grp = jc * 2 + t2
                        stg = opst.tile([P, 2, D], bf16, tag="opst", name=f"stg{grp}")
                        for t in range(2):
                            jt = grp * 2 + t
                            pso = [big.tile([P, JC], f32, tag="big",
                                            name=f"pso_{jt}_{ec}") for ec in range(2)]
                            for g in range(2):
                                for ec in range(2):
                                    nc.tensor.matmul(pso[ec], otp[g][:, bass.ts(jt, P)],
                                                     wo2[:, g, bass.ts(ec, JC)],
                                                     start=(g == 0), stop=(g == 1))
                                    if ec == 1:
                                        # same stationary as ec==0: skip reload
                                        mm = [i for bb in nc.m.functions[0].blocks
                                              for i in bb.instructions
                                              if type(i).__name__ == "InstMatmult"]
                                        mm[-1].ldweights = False
                            # split PSUM->SBUF stores across DVE and ACT so
                            # neither engine becomes the chunk-boundary choke
                            nc.vector.tensor_copy(out=stg[:, t, bass.ts(0, JC)],
                                                  in_=pso[0])
                            nc.scalar.activation(
                                out=stg[:, t, bass.ts(1, JC)], in_=pso[1],
                                func=mybir.ActivationFunctionType.Copy)
                        outap = out.ap().rearrange("(jt p) e -> p jt e", p=P)
                        if jc == NJC - 1 and t2 == 1:
                            # shorten the final drain: fire per-j-tile DMAs
                            for t in range(2):
                                nc.sync.dma_start(
                                    out=outap[:, grp * 2 + t:grp * 2 + t + 1, :],
                                    in_=stg[:, t:t + 1, :])
                        else:
                            nc.sync.dma_start(
                                out=outap[:, grp * 2:(grp + 1) * 2, :], in_=stg)

            if loop_k and loop_k > 1:
                with tc.For_i(0, loop_k, 1):
                    emit()
            else:
                emit()

    _legalize_waits(nc)
    return nc


def _mask_tiles():
    pp = np.arange(P)[:, None]
    ff = np.arange(P)[None, :]
    return np.where(pp <= ff, 0.0, MASKVAL).astype(np.float32)


def _make_in_maps(q, k, v, v_mask, a_mask, Wq, bq, Wk, bk, Wv, bv, Wo, causal):
    masks = _mask_tiles()
    am2 = np.asarray(a_mask).reshape(S, S).astype(bool)
    qTb = [np.ascontiguousarray(q[b].T.astype(npbf16)) for b in range(B)]
    kTb = [np.ascontiguousarray(k[b].T.astype(npbf16)) for b in range(B)]
    vTb = [np.ascontiguousarray(v[b].T.astype(npbf16)) for b in range(B)]
    in_maps = []
    for c in range(NCORES):
        b, hg = divmod(c, GROUPS)
        cs = slice(hg * CW, (hg + 1) * CW)
        im = {
            "qT": qTb[b],
            "kT": kTb[b],
            "vT": vTb[b],
            "wqkv": np.ascontiguousarray(
                np.stack([Wq[:, cs], Wk[:, cs], Wv[:, cs]], axis=0).astype(npbf16)),
            "wo": np.ascontiguousarray(Wo[cs, :].astype(npbf16)),
            "bqk": np.ascontiguousarray(
                np.stack([bq[cs].reshape(2, P), bk[cs].reshape(2, P)], axis=0)),
            "bv": np.ascontiguousarray(bv[cs]),
            "masks": masks,
        }
        if not causal:
            add = np.where(am2, 0.0, MASKVAL).astype(np.float32).T.copy()
            add += np.where(np.asarray(v_mask)[b], 0.0, MASKVAL).astype(np.float32)[:, None]
            im["amask"] = add
            im["masks"] = np.zeros_like(masks)
        in_maps.append(im)
    return in_maps


def kernel(q, k, v, q_mask, v_mask, a_mask, Wq, bq, Wk, bk, Wv, bv, Wo, bo):
    q = np.asarray(q, dtype=np.float32)
    k = np.asarray(k, dtype=np.float32)
    v = np.asarray(v, dtype=np.float32)
    q_mask = np.asarray(q_mask)
    v_mask = np.asarray(v_mask)
    a_mask = np.asarray(a_mask)
    Wq = np.asarray(Wq, dtype=np.float32); bq = np.asarray(bq, dtype=np.float32)
    Wk = np.asarray(Wk, dtype=np.float32); bk = np.asarray(bk, dtype=np.float32)
    Wv = np.asarray(Wv, dtype=np.float32); bv = np.asarray(bv, dtype=np.float32)
    Wo = np.asarray(Wo, dtype=np.float32); bo = np.asarray(bo, dtype=np.float32)

    am2 = a_mask.reshape(S, S).astype(bool)
    causal = bool((am2 == np.tril(np.ones((S, S), dtype=bool))).all()) \
        and bool(v_mask.all())

    if causal not in _CACHE:
        _CACHE[causal] = _build(causal=causal)
    nc = _CACHE[causal]

    in_maps = _make_in_maps(q, k, v, v_mask, a_mask, Wq, bq, Wk, bk, Wv, bv, Wo,
                            causal)
    res = bass2jax.run_bass_via_pjrt(nc, in_maps, n_cores=NCORES)

    outf = np.zeros((B, S, D), dtype=np.float32)
    for c in range(NCORES):
        b = c // GROUPS
        outf[b] += res[c]["out"].astype(np.float32)
    outf += bo[None, None, :]
    outf *= q_mask.astype(np.float32)[:, :, None]
    return outf


# revision 38
# speedup vs baseline: 1.2182x; 1.2182x over previous
"""Trainium2 Bass kernel for nn_MultiHeadAttention (B=2, S=2048, D=1024, H=16,
DK=DV=64, causal mask), sharded over 8 NeuronCores.

Sharding: data-parallel on batch (cores 0-3 -> b=0, cores 4-7 -> b=1) x
tensor-parallel on heads (each core owns 4 heads = 256 cols of Wq/Wk/Wv and
256 rows of Wo). Each core computes a partial output projection; the host sums
the 4 partials per batch, adds bo, and applies q_mask.

All matmuls run in bf16 (1 PE cycle/row vs 4 for fp32), accumulating in fp32
PSUM. HW charges ~1ns per stationary column for each self-loading matmul
(LD_WEIGHTS is serial and unmodeled by the cost model), so the kernel is
organized kt-MAJOR to share weight loads via ldweights=False chaining:

  1. Prologue: all q/k/v chunks stream in; Q/K projections for all 4 j-chunks
     with each weight tile loaded once per chunk-PAIR (jc pairs share the
     stationary; the second matmul skips its load).
  2. Attention per head h, kt-major: one kwt stationary load serves the
     score matmuls of every live j-chunk (A^T[k,j] tiles, diagonal tiles
     column-restricted with one triangular DVE mask add); exp on ACT
     (scale=1/8 folded, bf16 out); AV batches lag two kt so the PE never
     waits on exp: one vw_aug stationary load serves all live chunks'
     accumulations (ones column = softmax denominators). V projections for
     kt+2 are emitted as PE filler inside head 0's loop. Reciprocals issue
     the moment a chunk's accumulation stops; the rank-1 broadcast +
     normalize multiply run at the head boundary into head-PAIR buffers.
  3. Output projection TRANSPOSED (out[e, j], contract full 128-row head
     pairs): per 128-col e-tile, each wo2 stationary load serves all 4
     j-chunks; host transposes back. PSUM: score/vw tag 3 banks + po/outproj
     tag 4 + rank-1 1 = 8.
"""
import numpy as np
import ml_dtypes

import concourse.bass as bass
import concourse.mybir as mybir
from concourse.tile import TileContext
from concourse import bass2jax

# ---- problem constants (hardcoded per contract) ----
B, S, D = 2, 2048, 1024
H, DK, DV = 16, 64, 64
NCORES = 8
GROUPS = NCORES // B          # cores per batch = 4
HC = H // GROUPS              # heads per core = 4
CW = HC * DK                  # per-core width = 256
P = 128                       # partitions
JC = 512                      # j-chunk (moving free dim)
NJC = S // JC                 # 4
NKT = S // P                  # 16 k-tiles
NDT = D // P                  # 8 D-tiles
MASKVAL = -8.0e4              # pre-scale additive mask (=> logit -1e4)

f32 = mybir.dt.float32
bf16 = mybir.dt.bfloat16
npbf16 = ml_dtypes.bfloat16

_CACHE = {}


def _legalize_waits(nc, max_waits=1):
    """This walrus build accepts at most one on_wait per instruction; hoist
    extras onto same-engine NOPs inserted immediately before."""
    import bass_rust
    n = 0
    for f in nc.m.functions:
        for bb in f.blocks:
            insts = bb.instructions
            if not any(
                (inst.sync_info is not None and len(inst.sync_info.on_wait) > max_waits)
                for inst in insts
            ):
                continue
            out = []
            for inst in insts:
                si = inst.sync_info
                if si is not None and len(si.on_wait) > max_waits:
                    waits = list(si.on_wait)
                    for w in waits[:-max_waits]:
                        nop = mybir.InstNoOp(name=f"lwnop-{n}")
                        n += 1
                        nop.engine = inst.engine
                        nop.sync_info = bass_rust.SyncInfo(on_wait=[w], on_update=[])
                        out.append(nop)
                    inst.sync_info = bass_rust.SyncInfo(
                        on_wait=waits[-max_waits:], on_update=list(si.on_update)
                    )
                out.append(inst)
            bb.instructions = out
    return n


def _build(causal=True, loop_k=None):
    nc = bass.Bass(trn_type="TRN2", target_bir_lowering=False, debug=False)

    qT = nc.dram_tensor("qT", [D, S], bf16, kind="ExternalInput")
    kT = nc.dram_tensor("kT", [D, S], bf16, kind="ExternalInput")
    vT = nc.dram_tensor("vT", [D, S], bf16, kind="ExternalInput")
    wqkv = nc.dram_tensor("wqkv", [3, D, CW], bf16, kind="ExternalInput")
    wo = nc.dram_tensor("wo", [CW, D], bf16, kind="ExternalInput")
    bqk = nc.dram_tensor("bqk", [2, 2, P], f32, kind="ExternalInput")  # [q/k, hp, d]
    bvv = nc.dram_tensor("bv", [CW], f32, kind="ExternalInput")
    masks = nc.dram_tensor("masks", [P, P], f32, kind="ExternalInput")
    amask = None
    if not causal:
        amask = nc.dram_tensor("amask", [S, S], f32, kind="ExternalInput")
    out = nc.dram_tensor("out", [D, S], bf16, kind="ExternalOutput")  # out^T

    def live_jcs(kt):
        if not causal:
            return list(range(NJC))
        return [jc for jc in range(NJC) if 4 * jc + 3 >= kt]

    with TileContext(nc) as tc:
        with tc.tile_pool(name="const", bufs=1) as const, \
             tc.tile_pool(name="chunks", bufs=4) as chunks, \
             tc.tile_pool(name="pt", bufs=12) as ptp, \
             tc.tile_pool(name="small", bufs=4) as small, \
             tc.tile_pool(name="opst", bufs=2) as opst, \
             tc.tile_pool(name="amp", bufs=4) as amp, \
             tc.tile_pool(name="psA", bufs=3, space="PSUM") as psA, \
             tc.tile_pool(name="psB", bufs=4, space="PSUM") as psB, \
             tc.tile_pool(name="rbp", bufs=1, space="PSUM") as rbp:

            def emit():
                blocks = nc.m.functions[0].blocks

                def chain_ldw():
                    """Mark the just-emitted matmul non-self-loading (its
                    stationary is already in the PE array)."""
                    for bb in reversed(blocks):
                        if bb.instructions and \
                                type(bb.instructions[-1]).__name__ == "InstMatmult":
                            bb.instructions[-1].ldweights = False
                            return
                    raise AssertionError("no trailing matmul found")

                # ---------- input DMAs: q,k first (prologue projections),
                # v behind (consumed by JIT V-projection during head 0) -----
                wqkv_sb = const.tile([P, 3, NDT, CW], bf16, tag="wqkv")
                for w in (0, 1):
                    nc.scalar.dma_start(
                        out=wqkv_sb[:, w],
                        in_=wqkv.ap().rearrange("w (dt p) c -> p w dt c", p=P)[:, w])
                srcs = (qT, kT, vT)
                ch = {}
                for w in (0, 1, 2):
                    for jc in range(NJC):
                        t = chunks.tile([P, NDT, JC], bf16, tag=f"chunk{w}",
                                        name=f"ch{w}_{jc}")
                        nc.sync.dma_start(
                            out=t,
                            in_=srcs[w].ap().rearrange("(dt p) s -> p dt s", p=P)
                            [:, :, bass.ts(jc, JC)])
                        ch[(w, jc)] = t
                    if w == 0:
                        nc.scalar.dma_start(
                            out=wqkv_sb[:, 2],
                            in_=wqkv.ap().rearrange("w (dt p) c -> p w dt c", p=P)[:, 2])

                bqk_sb = const.tile([P, 2, 2], f32, tag="bqk")
                nc.scalar.dma_start(out=bqk_sb, in_=bqk.ap().rearrange("qk hp p -> p qk hp"))
                bv_sb = const.tile([P, CW], f32, tag="bv")
                nc.scalar.dma_start(out=bv_sb,
                                  in_=bass.AP(tensor=bvv, offset=0, ap=[[0, P], [1, CW]]))
                masks_sb = const.tile([P, P], f32, tag="masks")
                nc.scalar.dma_start(out=masks_sb, in_=masks.ap())
                wo2 = const.tile([P, 2, D], bf16, tag="wo2")
                nc.scalar.dma_start(out=wo2, in_=wo.ap().rearrange("(g p) e -> p g e", p=P))

                ones64 = const.tile([1, DV], bf16, tag="ones64")
                nc.vector.memset(ones64, 1.0)

                qwt = [const.tile([P, S], bf16, tag=f"qwt{hp}", name=f"qwt{hp}")
                       for hp in range(2)]
                kwt = [const.tile([P, S], bf16, tag=f"kwt{hp}", name=f"kwt{hp}")
                       for hp in range(2)]
                vw_aug = const.tile([P, NKT, HC, DV + 1], bf16, tag="vw_aug")
                nc.vector.memset(vw_aug[:, :, :, DV:DV + 1], 1.0)
                otp = [const.tile([P, S], bf16, tag=f"otp{g}", name=f"otp{g}")
                       for g in range(2)]

                # ---------- prologue: Q/K projections, weight loads shared
                # across chunk pairs ----------
                def proj_pair(w, hp, pair):
                    """One chunk-pair of the Q/K projection: the weight tile
                    loads once, the pair's second matmul chains.  Allocates
                    and frees its PSUM tiles contiguously, so it is safe to
                    emit as a filler between attention score groups."""
                    dst = qwt if w == 0 else kwt
                    jcs = (2 * pair, 2 * pair + 1)
                    pss = [psA.tile([P, JC], f32, tag="sc",
                                    name=f"pj{w}{hp}{jc}") for jc in jcs]
                    for dt in range(NDT):
                        for i, jc in enumerate(jcs):
                            nc.tensor.matmul(
                                pss[i], wqkv_sb[:, w, dt, bass.ts(hp, P)],
                                ch[(w, jc)][:, dt, :],
                                start=(dt == 0), stop=(dt == NDT - 1))
                            if i > 0:
                                chain_ldw()
                    for i, jc in enumerate(jcs):
                        nc.vector.tensor_scalar_add(
                            dst[hp][:, bass.ts(jc, JC)], pss[i],
                            bqk_sb[:, w, hp:hp + 1])

                # prologue: only the hp0 projections (heads 0-1); hp1 runs as
                # PE filler inside head 1's loop (heads 2-3 need it)
                for w in (0, 1):
                    for pair in range(2):
                        proj_pair(w, 0, pair)
                hp1_fill = [(w, pair) for w in (0, 1) for pair in range(2)]

                # V projection for one k-tile, [k, c] layout + bias + ones col
                def emit_vw(kt):
                    jc, t = divmod(kt, JC // P)
                    pv = psA.tile([P, CW], f32, tag="sc", name=f"pv{kt}")
                    chv = ch[(2, jc)]
                    for dt in range(NDT):
                        nc.tensor.matmul(pv, chv[:, dt, bass.ts(t, P)],
                                         wqkv_sb[:, 2, dt, :],
                                         start=(dt == 0), stop=(dt == NDT - 1))
                    nc.vector.tensor_add(vw_aug[:, kt, :, 0:DV], pv, bv_sb)

                emit_vw(0)
                emit_vw(1)

                # ---------- attention, kt-major per head ----------
                AVLAG = 2
                for h in range(HC):
                    hp, hh = divmod(h, 2)
                    drow = slice(hh * DV, hh * DV + DV)
                    g = h // 2
                    po = {jc: psB.tile([DV + 1, JC], f32, tag="av",
                                       name=f"po_{h}_{jc}")
                          for jc in range(NJC)}
                    rcps = {}
                    avq = {}     # kt -> [(jc, pt, off), ...]

                    def av_batch(ktq):
                        items = avq.pop(ktq, [])
                        for i, (jc, ptile, off) in enumerate(items):
                            last = causal and (ktq == 4 * jc + 3)
                            if not causal:
                                last = ktq == NKT - 1
                            nc.tensor.matmul(po[jc][:, off:],
                                             vw_aug[:, ktq, h, :], ptile[:, off:],
                                             start=(ktq == 0), stop=last,
                                             skip_group_check=True)
                            if i > 0:
                                chain_ldw()
                            if last:
                                rcp = small.tile([1, JC], bf16, tag="rcp",
                                                 name=f"rcp{h}{jc}")
                                with nc.allow_low_precision(reason="bf16 recip"):
                                    nc.vector.reciprocal(rcp, po[jc][DV:DV + 1, :])
                                rcps[jc] = rcp

                    for kt in range(NKT):
                        jcs = live_jcs(kt)
                        # mask/exp are emitted right after each chunk's score
                        # so the PSUM slot's reader exists before the slot
                        # recycles; DVE/ACT ops between the score matmuls do
                        # not disturb the PE array, so the ldweights chain
                        # across the chunks stays valid.
                        for i, jc in enumerate(jcs):
                            dlt = kt - 4 * jc
                            off = P * dlt if (causal and dlt > 0) else 0
                            ps = psA.tile([P, JC], f32, tag="sc",
                                          name=f"sc_{h}_{kt}_{jc}")
                            nc.tensor.matmul(
                                ps[:, off:], kwt[hp][drow, bass.ts(kt, P)],
                                qwt[hp][drow, jc * JC + off:(jc + 1) * JC],
                                start=True, stop=True)
                            if i > 0:
                                chain_ldw()
                            if causal and 0 <= dlt <= 3:
                                nc.vector.tensor_add(ps[:, off:off + P],
                                                     ps[:, off:off + P], masks_sb)
                            if not causal:
                                am = amp.tile([P, JC], f32, tag="am",
                                              name=f"am_{h}_{kt}_{jc}")
                                nc.sync.dma_start(
                                    out=am,
                                    in_=amask.ap()[bass.ts(kt, P), bass.ts(jc, JC)])
                                nc.vector.tensor_add(ps, ps, am)
                            pt = ptp.tile([P, JC], bf16, tag="pt",
                                          name=f"pt_{h}_{kt}_{jc}")
                            nc.scalar.activation(out=pt[:, off:], in_=ps[:, off:],
                                                 func=mybir.ActivationFunctionType.Exp,
                                                 scale=0.125)
                            avq.setdefault(kt, []).append((jc, pt, off))
                        if kt >= AVLAG:
                            av_batch(kt - AVLAG)
                        # PE fillers: JIT V-projection two k-tiles ahead
                        # (head 0), hp1 projections spread over head 1
                        if h == 0 and kt + AVLAG < NKT:
                            emit_vw(kt + AVLAG)
                        if h == 1 and kt in (2, 5, 8, 11) and hp1_fill:
                            w, pair = hp1_fill.pop(0)
                            proj_pair(w, 1, pair)
                    for ktq in range(NKT - AVLAG, NKT):
                        av_batch(ktq)

                    # normalize: rank-1 broadcast of 1/denominator, multiply
                    # into head-pair buffers
                    for jc in range(NJC):
                        rb = rbp.tile([DV, JC], f32, tag="rb", name=f"rb{h}{jc}")
                        nc.tensor.matmul(rb, ones64, rcps[jc], start=True,
                                         stop=True, skip_group_check=True)
                        rbs = small.tile([DV, JC], f32, tag="rbs",
                                         name=f"rbs{h}{jc}")
                        nc.vector.tensor_copy(out=rbs, in_=rb)
                        nc.vector.tensor_mul(
                            otp[g][hh * DV:(hh + 1) * DV, bass.ts(jc, JC)],
                            po[jc][0:DV, :], rbs)

                # ---------- output projection, transposed (out[e, j]) ----
                outap = out.ap().rearrange("(et p) s -> p et s", p=P)
                for et in range(NDT):
                    pss = [psB.tile([P, JC], f32, tag="av", name=f"op{et}{jc}")
                           for jc in range(NJC)]
                    for gg in range(2):
                        for jc in range(NJC):
                            nc.tensor.matmul(pss[jc], wo2[:, gg, bass.ts(et, P)],
                                             otp[gg][:, bass.ts(jc, JC)],
                                             start=(gg == 0), stop=(gg == 1))
                            if jc > 0:
                                chain_ldw()
                    stg = opst.tile([P, S], bf16, tag="opst", name=f"stg{et}")
                    for jc in range(NJC):
                        if jc % 2 == 0:
                            nc.vector.tensor_copy(out=stg[:, bass.ts(jc, JC)],
                                                  in_=pss[jc])
                        else:
                            nc.scalar.activation(
                                out=stg[:, bass.ts(jc, JC)], in_=pss[jc],
                                func=mybir.ActivationFunctionType.Copy)
                    nc.sync.dma_start(out=outap[:, et:et + 1, :], in_=stg)

            if loop_k and loop_k > 1:
                with tc.For_i(0, loop_k, 1):
                    emit()
            else:
                emit()

    _legalize_waits(nc)
    return nc


def _mask_tiles():
    pp = np.arange(P)[:, None]
    ff = np.arange(P)[None, :]
    return np.where(pp <= ff, 0.0, MASKVAL).astype(np.float32)


def _make_in_maps(q, k, v, v_mask, a_mask, Wq, bq, Wk, bk, Wv, bv, Wo, causal):
    masks = _mask_tiles()
    am2 = np.asarray(a_mask).reshape(S, S).astype(bool)
    qTb = [np.ascontiguousarray(q[b].T.astype(npbf16)) for b in range(B)]
    kTb = [np.ascontiguousarray(k[b].T.astype(npbf16)) for b in range(B)]
    vTb = [np.ascontiguousarray(v[b].T.astype(npbf16)) for b in range(B)]
    in_maps = []
    for c in range(NCORES):
        b, hg = divmod(c, GROUPS)
        cs = slice(hg * CW, (hg + 1) * CW)
        im = {
            "qT": qTb[b],
            "kT": kTb[b],
            "vT": vTb[b],
            "wqkv": np.ascontiguousarray(
                np.stack([Wq[:, cs], Wk[:, cs], Wv[:, cs]], axis=0).astype(npbf16)),
            "wo": np.ascontiguousarray(Wo[cs, :].astype(npbf16)),
            "bqk": np.ascontiguousarray(
                np.stack([bq[cs].reshape(2, P), bk[cs].reshape(2, P)], axis=0)),
            "bv": np.ascontiguousarray(bv[cs]),
            "masks": masks,
        }
        if not causal:
            add = np.where(am2, 0.0, MASKVAL).astype(np.float32).T.copy()
            add += np.where(np.asarray(v_mask)[b], 0.0, MASKVAL).astype(np.float32)[:, None]
            im["amask"] = add
            im["masks"] = np.zeros_like(masks)
        in_maps.append(im)
    return in_maps


def kernel(q, k, v, q_mask, v_mask, a_mask, Wq, bq, Wk, bk, Wv, bv, Wo, bo):
    q = np.asarray(q, dtype=np.float32)
    k = np.asarray(k, dtype=np.float32)
    v = np.asarray(v, dtype=np.float32)
    q_mask = np.asarray(q_mask)
    v_mask = np.asarray(v_mask)
    a_mask = np.asarray(a_mask)
    Wq = np.asarray(Wq, dtype=np.float32); bq = np.asarray(bq, dtype=np.float32)
    Wk = np.asarray(Wk, dtype=np.float32); bk = np.asarray(bk, dtype=np.float32)
    Wv = np.asarray(Wv, dtype=np.float32); bv = np.asarray(bv, dtype=np.float32)
    Wo = np.asarray(Wo, dtype=np.float32); bo = np.asarray(bo, dtype=np.float32)

    am2 = a_mask.reshape(S, S).astype(bool)
    causal = bool((am2 == np.tril(np.ones((S, S), dtype=bool))).all()) \
        and bool(v_mask.all())

    if causal not in _CACHE:
        _CACHE[causal] = _build(causal=causal)
    nc = _CACHE[causal]

    in_maps = _make_in_maps(q, k, v, v_mask, a_mask, Wq, bq, Wk, bk, Wv, bv, Wo,
                            causal)
    res = bass2jax.run_bass_via_pjrt(nc, in_maps, n_cores=NCORES)

    outf = np.zeros((B, S, D), dtype=np.float32)
    for c in range(NCORES):
        b = c // GROUPS
        outf[b] += res[c]["out"].astype(np.float32).T
    outf += bo[None, None, :]
    outf *= q_mask.astype(np.float32)[:, :, None]
    return outf
